# revision 1
# baseline (speedup 1.0000x reference)
"""Trainium2 Bass kernel for nn_DiagonalTraining (anti-diagonal per-diag Linear).

out[b, r, c] = sum_{r'} W[d, r - r0(d), r' - r0(d)] * x[b, r', d - r'] + bias,
with d = r + c, over the valid range of r' for diagonal d.

Strategy: shard the 511 independent diagonals across 8 cores (expert-style).
The host packs each core's work into uniform-shape matmul jobs:
  - short diagonals (n <= 128): pair-packed into bins of K=128 (block-diag W),
    one matmul [K=128] x [N=128] per bin, 17 bins/core.
  - long diagonals (128 < n <= 256): one job each, PSUM-accumulated over 2
    K-chunks of 128, N=256 outputs, 32 jobs/core.
Stationary operand = gathered diagonal data xd^T [K, batch=128]; moving
operand = per-diagonal weights [K, N]. PSUM out = [batch=128, N].
Host scatters the packed outputs back to the grid and adds bias.
"""

import sys

sys.path.insert(0, "/opt/trn_rl_repo")

import numpy as np

B, S = 128, 256
D = 2 * S - 1  # 511
NCORES = 8
NSB = 17  # short-diagonal bins per core
NLJ = 32  # long-diagonal jobs per core

USE_BF16 = False  # flipped after precision/perf measurement
USE_F32R = True  # float32r: same fp32 bits, full-rate PE streaming at N>=256
TRACE = False  # test.py sets True to pull exec_time_ns from the NTFF profile
last_results = None


def _geom(d):
    r0 = max(0, d - S + 1)
    n = d + 1 if d < S else 2 * S - 1 - d
    return r0, n


def _job_tables():
    """Static per-core packing tables (indices + masks + scatter targets)."""
    # ---- short bins: 129 real bins + 7 dummies = 136 = 8 * 17
    sbins = []
    for kk in range(1, 64):
        sbins.append([kk - 1, 127 - kk])
        sbins.append([511 - kk, 383 + kk])
    sbins.append([63, 447])
    sbins.append([127])
    sbins.append([383])
    sbins += [[] for _ in range(136 - len(sbins))]
    # ---- long jobs: d in [128, 382] (255) + 1 dummy = 256 = 8 * 32
    ljobs = [[d] for d in range(128, 383)] + [[]]

    cores = []
    for c in range(NCORES):
        my_s = sbins[c::NCORES]
        my_l = ljobs[c::NCORES]
        xds_i = np.zeros((NSB, 128), np.int64)
        xds_m = np.zeros((NSB, 128), np.float32)
        ws_i = np.zeros((NSB, 128, 128), np.int64)
        ws_m = np.zeros((NSB, 128, 128), np.float32)
        tgt_s = np.full((NSB, 128), -1, np.int64)
        for j, bin_ds in enumerate(my_s):
            off = 0
            for d in bin_ds:
                r0, n = _geom(d)
                i = np.arange(n)
                r = r0 + i
                col = d - r
                xds_i[j, off : off + n] = r * S + col
                xds_m[j, off : off + n] = 1.0
                # W[d, m, k] at [k, m] (k = contraction pos, m = output pos)
                ws_i[j, off : off + n, off : off + n] = (
                    d * S * S + i[None, :] * S + i[:, None]
                )
                ws_m[j, off : off + n, off : off + n] = 1.0
                tgt_s[j, off : off + n] = r * S + col
                off += n

        xdl_i = np.zeros((NLJ, 2, 128), np.int64)
        xdl_m = np.zeros((NLJ, 2, 128), np.float32)
        wl_i = np.zeros((NLJ, 2, 128, 256), np.int64)
        wl_m = np.zeros((NLJ, 2, 128, 256), np.float32)
        tgt_l = np.full((NLJ, 256), -1, np.int64)
        for j, job in enumerate(my_l):
            if not job:
                continue
            (d,) = job
            r0, n = _geom(d)
            m = np.arange(256)
            for ch in range(2):
                i = ch * 128 + np.arange(128)
                v = i < n
                r = r0 + np.minimum(i, n - 1)
                xdl_i[j, ch] = (r * S + (d - r)) * v
                xdl_m[j, ch] = v.astype(np.float32)
                mv = (m < n)[None, :] & v[:, None]
                wl_i[j, ch] = (d * S * S + np.minimum(m, n - 1)[None, :] * S + np.minimum(i, n - 1)[:, None]) * mv
                wl_m[j, ch] = mv.astype(np.float32)
            mr = r0 + m[: n]
            tgt_l[j, :n] = mr * S + (d - mr)
        cores.append(
            dict(
                xds_i=xds_i, xds_m=xds_m, ws_i=ws_i, ws_m=ws_m, tgt_s=tgt_s,
                xdl_i=xdl_i, xdl_m=xdl_m, wl_i=wl_i, wl_m=wl_m, tgt_l=tgt_l,
            )
        )
    # bias gather: out_flat[p] += b[d, r - r0(d)] for p = r*S + c, d = r + c
    rr, cc = np.divmod(np.arange(S * S), S)
    dd = rr + cc
    r0v = np.maximum(0, dd - S + 1)
    bidx = dd * S + (rr - r0v)
    return cores, bidx


_TABLES = None
_PROG = {}


def _tables():
    global _TABLES
    if _TABLES is None:
        _TABLES = _job_tables()
    return _TABLES


def _build_program(use_bf16):
    import concourse.bass as bass
    import concourse.mybir as mybir
    import concourse.tile as tile

    f32 = mybir.dt.float32
    if use_bf16:
        dt_in = mybir.dt.bfloat16
    elif USE_F32R:
        dt_in = mybir.dt.float32r
    else:
        dt_in = f32
    nc = bass.Bass()
    bl = nc.dram_tensor("bl", [128, NLJ * 2 * 384], dt_in, kind="ExternalInput")
    bs = nc.dram_tensor("bs", [128, NSB * 256], dt_in, kind="ExternalInput")
    ys = nc.dram_tensor("ys", [128, NSB * 128], f32, kind="ExternalOutput")
    yl = nc.dram_tensor("yl", [128, NLJ * 256], f32, kind="ExternalOutput")

    CH = 4  # L-jobs per load group
    NPS = 6  # psum slots (full banks, cycled)
    SG_BOUNDS = [(0, 8), (8, NSB)]  # S-bin load groups

    # SBUF staging (no reuse -> no WAR deps on input DMAs)
    BTL = [
        nc.alloc_sbuf_tensor(f"btl{g}", [128, CH * 2 * 384], dt_in).ap()
        for g in range(NLJ // CH)
    ]
    BTS = [
        nc.alloc_sbuf_tensor(f"bts{g}", [128, (j1 - j0) * 256], dt_in).ap()
        for g, (j0, j1) in enumerate(SG_BOUNDS)
    ]
    YL = nc.alloc_sbuf_tensor("YL", [128, NLJ * 256], f32).ap()
    YS = nc.alloc_sbuf_tensor("YS", [128, NSB * 128], f32).ap()
    PS = [
        nc.alloc_psum_tensor(f"ps{i}", [128, 512], f32).ap() for i in range(NPS)
    ]

    # unified job list: (required_input_dma_count, n_chunks, lhs/rhs slices, out)
    jobs = []
    for j in range(NLJ):
        g = j // CH
        jj = j % CH
        ops = []
        for ch in range(2):
            o = (jj * 2 + ch) * 384
            ops.append((BTL[g], o))
        jobs.append(("L", g + 1, ops, j))
    n_l_dma = NLJ // CH
    for gi, (j0, j1) in enumerate(SG_BOUNDS):
        for j in range(j0, j1):
            o = (j - j0) * 256
            jobs.append(("S", n_l_dma + gi + 1, [(BTS[gi], o)], j))

    DIN = [
        nc.alloc_semaphore(f"din{i}")
        for i in range(NLJ // CH + len(SG_BOUNDS))
    ]  # one per input DMA (completion order across queues is not FIFO)
    P = nc.alloc_semaphore("P")  # PE job completions
    C = nc.alloc_semaphore("C")  # DVE copy completions
    DO = nc.alloc_semaphore("DO")  # output DMA completions (x16)

    with nc.Block() as block:

        @block.sync
        def _(sync):
            for g in range(n_l_dma):
                sync.dma_start(
                    out=BTL[g][:], in_=bl[:, g * CH * 2 * 384 : (g + 1) * CH * 2 * 384]
                ).then_inc(DIN[g], 16)
            for gi, (j0, j1) in enumerate(SG_BOUNDS):
                sync.dma_start(
                    out=BTS[gi][:], in_=bs[:, j0 * 256 : j1 * 256]
                ).then_inc(DIN[n_l_dma + gi], 16)
            n_out = 0
            for g in range(n_l_dma):
                sync.wait_ge(C, (g + 1) * CH)
                sync.dma_start(
                    out=yl[:, g * CH * 256 : (g + 1) * CH * 256],
                    in_=YL[:, g * CH * 256 : (g + 1) * CH * 256],
                ).then_inc(DO, 16)
                n_out += 1
            for gi, (j0, j1) in enumerate(SG_BOUNDS):
                sync.wait_ge(C, NLJ + j1)
                sync.dma_start(
                    out=ys[:, j0 * 128 : j1 * 128], in_=YS[:, j0 * 128 : j1 * 128]
                ).then_inc(DO, 16)
                n_out += 1
            sync.wait_ge(DO, 16 * n_out)

        @block.tensor
        def _(tensor):
            cur_d = 0
            for ji, (kind, dthr, ops, j) in enumerate(jobs):
                if dthr > cur_d:
                    tensor.wait_ge(DIN[dthr - 1], 16)
                    cur_d = dthr
                if ji >= NPS:
                    tensor.wait_ge(C, ji - NPS + 1)
                ps = PS[ji % NPS]
                if kind == "L":
                    for ch, (bt, o) in enumerate(ops):
                        mm = nc.tensor.matmul(
                            ps[:, 0:256],
                            bt[:, o : o + 128],
                            bt[:, o + 128 : o + 384],
                            start=(ch == 0),
                            stop=(ch == 1),
                        )
                else:
                    (bt, o) = ops[0]
                    mm = nc.tensor.matmul(
                        ps[:, 0:128],
                        bt[:, o : o + 128],
                        bt[:, o + 128 : o + 256],
                        start=True,
                        stop=True,
                    )
                mm.then_inc(P, 1)

        @block.vector
        def _(vector):
            for ji, (kind, dthr, ops, j) in enumerate(jobs):
                vector.wait_ge(P, ji + 1)
                ps = PS[ji % NPS]
                if kind == "L":
                    cp = nc.vector.tensor_copy(
                        YL[:, j * 256 : (j + 1) * 256], ps[:, 0:256]
                    )
                else:
                    cp = nc.vector.tensor_copy(
                        YS[:, j * 128 : (j + 1) * 128], ps[:, 0:128]
                    )
                cp.then_inc(C, 1)

    return nc


def _get_program(use_bf16):
    if use_bf16 not in _PROG:
        _PROG[use_bf16] = _build_program(use_bf16)
    return _PROG[use_bf16]


def _pack_core(t, x_flat, W_flat, np_dt):
    xds = (x_flat[:, t["xds_i"]] * t["xds_m"]).astype(np_dt)  # [B, NSB, 128]
    XDS = xds.transpose(2, 1, 0)  # [128k, NSB, 128b]
    ws = (W_flat[t["ws_i"]] * t["ws_m"]).astype(np_dt)  # [NSB, 128k, 128m]
    WS = ws.transpose(1, 0, 2)  # [128k, NSB, 128m]
    BS = np.concatenate([XDS, WS], axis=2).reshape(128, NSB * 256)
    xdl = (x_flat[:, t["xdl_i"]] * t["xdl_m"]).astype(np_dt)  # [B, NLJ, 2, 128]
    XDL = xdl.transpose(3, 1, 2, 0).reshape(128, NLJ * 2, 128)
    wldat = (W_flat[t["wl_i"]] * t["wl_m"]).astype(np_dt)  # [NLJ, 2, 128, 256]
    WL = wldat.transpose(2, 0, 1, 3).reshape(128, NLJ * 2, 256)
    BL = np.concatenate([XDL, WL], axis=2).reshape(128, NLJ * 2 * 384)
    return {
        "bl": np.ascontiguousarray(BL),
        "bs": np.ascontiguousarray(BS),
    }


def kernel(x, W, b):
    import ml_dtypes
    from concourse.bass_utils import run_bass_kernel_spmd

    x = np.asarray(x, np.float32)
    W = np.asarray(W, np.float32)
    b = np.asarray(b, np.float32)
    cores, bidx = _tables()
    np_dt = ml_dtypes.bfloat16 if USE_BF16 else np.float32
    x_flat = x.reshape(B, S * S)
    W_flat = W.reshape(-1)
    in_maps = [_pack_core(t, x_flat, W_flat, np_dt) for t in cores]
    nc = _get_program(USE_BF16)
    res = run_bass_kernel_spmd(
        nc, in_maps, core_ids=list(range(NCORES)), trace=TRACE
    )
    global last_results
    last_results = res
    out_flat = np.zeros((B, S * S), np.float32)
    for c, t in enumerate(cores):
        ysv = res.results[c]["ys"].reshape(B, -1)
        ylv = res.results[c]["yl"].reshape(B, -1)
        fs = t["tgt_s"].reshape(-1)
        vs = fs >= 0
        out_flat[:, fs[vs]] = ysv[:, vs]
        fl = t["tgt_l"].reshape(-1)
        vl = fl >= 0
        out_flat[:, fl[vl]] = ylv[:, vl]
    out_flat += b.reshape(-1)[bidx][None, :]
    return out_flat.reshape(B, S, S)



# revision 2
# speedup vs baseline: 1.6722x; 1.6722x over previous
"""Trainium2 Bass kernel for nn_DiagonalTraining (anti-diagonal per-diag Linear).

out[b, r, c] = sum_{r'} W[d, r - r0(d), r' - r0(d)] * x[b, r', d - r'] + bias,
with d = r + c, over the valid range of r' for diagonal d.

Strategy: shard the 511 independent diagonals across 8 cores (expert-style),
all data in bf16 (halves DMA traffic vs f32; bf16 matmuls stream 1 col/cycle
at any N, unlike f32r which needs N>=256).
  - long diagonals (n > 128): sorted by n descending, assigned round-robin so
    slot j has ~equal n on every core (SPMD shares one program). Slot matmul
    shapes use the slot max: PSUM[128b, N_j] accumulated over K-chunks
    (128, N_j-128); W shipped as [K, N_j] exact-width columns.
  - short diagonals (n <= 128): pair-packed into bins of K=128 (block-diag W),
    one matmul [K=128] x [N=128] per bin, 17 bins/core.
Stationary operand = gathered diagonal data xd^T [K, batch=128]; moving
operand = per-diagonal weights [K, N]. PSUM out = [batch=128, N] -> bf16.
Host scatters the packed outputs back to the grid and adds bias.
"""

import sys

sys.path.insert(0, "/opt/trn_rl_repo")

import numpy as np

B, S = 128, 256
D = 2 * S - 1  # 511
NCORES = 8
NSB = 17  # short-diagonal bins per core
NLJ = 32  # long-diagonal slots per core

TRACE = False  # test.py sets True to pull exec_time_ns from the NTFF profile
last_results = None


def _geom(d):
    r0 = max(0, d - S + 1)
    n = d + 1 if d < S else 2 * S - 1 - d
    return r0, n


def _slot_geom():
    """Long diags sorted by n desc, round-robin to cores; slot-wise max n."""
    longs = [(d, _geom(d)[1]) for d in range(128, 383)]
    longs.sort(key=lambda t: (-t[1], t[0]))
    per_core = [[None] * NLJ for _ in range(NCORES)]
    slot_n = [0] * NLJ
    for i, (d, n) in enumerate(longs):
        c, j = i % NCORES, i // NCORES
        per_core[c][j] = d
        slot_n[j] = max(slot_n[j], n)
    return per_core, slot_n


def _job_tables():
    """Static per-core packing tables (indices + masks + scatter targets)."""
    # ---- short bins: 129 real bins + 7 dummies = 136 = 8 * 17
    sbins = []
    for kk in range(1, 64):
        sbins.append([kk - 1, 127 - kk])
        sbins.append([511 - kk, 383 + kk])
    sbins.append([63, 447])
    sbins.append([127])
    sbins.append([383])
    sbins += [[] for _ in range(136 - len(sbins))]

    long_per_core, slot_n = _slot_geom()
    lwoff = np.concatenate([[0], np.cumsum([2 * n for n in slot_n])])
    yoff = np.concatenate([[0], np.cumsum(slot_n)])
    LWC = int(lwoff[-1])  # long W cols per core
    LYC = int(yoff[-1])  # long out cols per core

    cores = []
    for c in range(NCORES):
        my_s = sbins[c::NCORES]
        xds_i = np.zeros((NSB, 128), np.int64)
        xds_m = np.zeros((NSB, 128), np.float32)
        ws_i = np.zeros((NSB, 128, 128), np.int64)
        ws_m = np.zeros((NSB, 128, 128), np.float32)
        tgt_s = np.full((NSB, 128), -1, np.int64)
        for j, bin_ds in enumerate(my_s):
            off = 0
            for d in bin_ds:
                r0, n = _geom(d)
                i = np.arange(n)
                r = r0 + i
                col = d - r
                xds_i[j, off : off + n] = r * S + col
                xds_m[j, off : off + n] = 1.0
                # W[d, m, k] at [k, m] (k = contraction pos, m = output pos)
                ws_i[j, off : off + n, off : off + n] = (
                    d * S * S + i[None, :] * S + i[:, None]
                )
                ws_m[j, off : off + n, off : off + n] = 1.0
                tgt_s[j, off : off + n] = r * S + col
                off += n

        xdl_i = np.zeros((NLJ, 2, 128), np.int64)
        xdl_m = np.zeros((NLJ, 2, 128), np.float32)
        wl_i = np.zeros((128, LWC), np.int64)
        wl_m = np.zeros((128, LWC), np.float32)
        tgt_l = np.full(LYC, -1, np.int64)
        for j in range(NLJ):
            d = long_per_core[c][j]
            if d is None:
                continue
            r0, n = _geom(d)
            N = slot_n[j]
            m = np.arange(N)
            mv = m < n
            for ch in range(2):
                i = ch * 128 + np.arange(128)
                v = i < n
                r = r0 + np.minimum(i, n - 1)
                xdl_i[j, ch] = (r * S + (d - r)) * v
                xdl_m[j, ch] = v.astype(np.float32)
                # W chunk ch at cols [lwoff[j] + ch*N, ...): [k-part, m-col]
                wm = mv[None, :] & v[:, None]
                wi = (
                    d * S * S
                    + np.minimum(m, n - 1)[None, :] * S
                    + np.minimum(i, n - 1)[:, None]
                )
                o = int(lwoff[j]) + ch * N
                wl_i[:, o : o + N] = wi * wm
                wl_m[:, o : o + N] = wm.astype(np.float32)
            mr = r0 + m[:n]
            tgt_l[int(yoff[j]) : int(yoff[j]) + n] = mr * S + (d - mr)
        cores.append(
            dict(
                xds_i=xds_i, xds_m=xds_m, ws_i=ws_i, ws_m=ws_m, tgt_s=tgt_s,
                xdl_i=xdl_i, xdl_m=xdl_m, wl_i=wl_i, wl_m=wl_m, tgt_l=tgt_l,
            )
        )
    # bias gather: out_flat[p] += b[d, r - r0(d)] for p = r*S + c, d = r + c
    rr, cc = np.divmod(np.arange(S * S), S)
    dd = rr + cc
    r0v = np.maximum(0, dd - S + 1)
    bidx = dd * S + (rr - r0v)
    return cores, bidx, slot_n, lwoff, yoff


_TABLES = None
_PROG = None


def _tables():
    global _TABLES
    if _TABLES is None:
        _TABLES = _job_tables()
    return _TABLES


# Input-DMA groups over long slots, then one group for the short bins.
LGROUPS = [(0, 11), (11, 22), (22, 32)]
# Output-DMA groups: job-count thresholds filled in by _build_program.


def _build_program(slot_n, lwoff, yoff):
    import concourse.bass as bass
    import concourse.mybir as mybir

    f32 = mybir.dt.float32
    bf16 = mybir.dt.bfloat16
    LWC = int(lwoff[-1])
    LYC = int(yoff[-1])
    SYC = NSB * 128
    # per-group input column counts: xd cols + W cols
    gcols = []
    for (j0, j1) in LGROUPS:
        gcols.append((j1 - j0) * 256 + int(lwoff[j1] - lwoff[j0]))
    gcols.append(NSB * 128 + NSB * 128)  # shorts: xd + W
    NG = len(gcols)

    nc = bass.Bass()
    bin_t = nc.dram_tensor("bin", [128, sum(gcols)], bf16, kind="ExternalInput")
    yout = nc.dram_tensor("yout", [128, LYC + SYC], bf16, kind="ExternalOutput")

    NPS = 6  # psum slots (full banks, cycled)

    BT = []
    goff = 0
    for g, gc in enumerate(gcols):
        BT.append(nc.alloc_sbuf_tensor(f"bt{g}", [128, gc], bf16).ap())
        goff += gc
    YB = nc.alloc_sbuf_tensor("YB", [128, LYC + SYC], bf16).ap()
    PS = [
        nc.alloc_psum_tensor(f"ps{i}", [128, 512], f32).ap() for i in range(NPS)
    ]

    # unified job list: (group, kind, j)
    jobs = []
    for g, (j0, j1) in enumerate(LGROUPS):
        for j in range(j0, j1):
            jobs.append((g, "L", j))
    for j in range(NSB):
        jobs.append((NG - 1, "S", j))

    DIN = [nc.alloc_semaphore(f"din{i}") for i in range(NG)]
    P = nc.alloc_semaphore("P")  # PE job completions
    C = nc.alloc_semaphore("C")  # DVE copy completions
    DO = nc.alloc_semaphore("DO")  # output DMA completions (x16)

    # output DMA groups: (jobs_done_threshold, col_start, col_end)
    n_long_jobs = NLJ
    og = [
        (LGROUPS[1][1], 0, int(yoff[LGROUPS[1][1]])),
        (n_long_jobs, int(yoff[LGROUPS[1][1]]), LYC),
        (n_long_jobs + NSB, LYC, LYC + SYC),
    ]

    with nc.Block() as block:

        @block.sync
        def _(sync):
            goff = 0
            for g, gc in enumerate(gcols):
                sync.dma_start(
                    out=BT[g][:], in_=bin_t[:, goff : goff + gc]
                ).then_inc(DIN[g], 16)
                goff += gc
            for thr, c0, c1 in og:
                sync.wait_ge(C, thr)
                sync.dma_start(
                    out=yout[:, c0:c1], in_=YB[:, c0:c1]
                ).then_inc(DO, 16)
            sync.wait_ge(DO, 16 * len(og))

        @block.tensor
        def _(tensor):
            cur_g = -1
            for ji, (g, kind, j) in enumerate(jobs):
                if g > cur_g:
                    tensor.wait_ge(DIN[g], 16)
                    cur_g = g
                if ji >= NPS:
                    tensor.wait_ge(C, ji - NPS + 1)
                ps = PS[ji % NPS]
                if kind == "L":
                    j0 = LGROUPS[g][0]
                    N = slot_n[j]
                    K2 = N - 128
                    xo = (j - j0) * 256
                    wo = (LGROUPS[g][1] - j0) * 256 + int(lwoff[j] - lwoff[j0])
                    bt = BT[g]
                    nc.tensor.matmul(
                        ps[:, 0:N],
                        bt[:, xo : xo + 128],
                        bt[:, wo : wo + N],
                        start=True,
                        stop=False,
                    )
                    mm = nc.tensor.matmul(
                        ps[:, 0:N],
                        bt[0:K2, xo + 128 : xo + 256],
                        bt[0:K2, wo + N : wo + 2 * N],
                        start=False,
                        stop=True,
                    )
                else:
                    bt = BT[NG - 1]
                    xo = j * 128
                    wo = NSB * 128 + j * 128
                    mm = nc.tensor.matmul(
                        ps[:, 0:128],
                        bt[:, xo : xo + 128],
                        bt[:, wo : wo + 128],
                        start=True,
                        stop=True,
                    )
                mm.then_inc(P, 1)

        @block.vector
        def _(vector):
            for ji, (g, kind, j) in enumerate(jobs):
                vector.wait_ge(P, ji + 1)
                ps = PS[ji % NPS]
                if kind == "L":
                    N = slot_n[j]
                    yo = int(yoff[j])
                    cp = nc.vector.tensor_copy(YB[:, yo : yo + N], ps[:, 0:N])
                else:
                    yo = LYC + j * 128
                    cp = nc.vector.tensor_copy(
                        YB[:, yo : yo + 128], ps[:, 0:128]
                    )
                cp.then_inc(C, 1)

    return nc


def _get_program():
    global _PROG
    if _PROG is None:
        _, _, slot_n, lwoff, yoff = _tables()
        _PROG = _build_program(slot_n, lwoff, yoff)
    return _PROG


def _pack_core(t, x_flat, W_flat, slot_n, lwoff, np_dt):
    # long xd: [B, NLJ, 2, 128] -> [128k, NLJ*2, B] col blocks
    xdl = x_flat[:, t["xdl_i"]] * t["xdl_m"]
    XDL = xdl.transpose(3, 1, 2, 0).reshape(128, NLJ * 2 * 128)
    WL = W_flat[t["wl_i"]] * t["wl_m"]  # [128, LWC]
    # short xd: [B, NSB, 128] -> [128k, NSB, B]
    xds = x_flat[:, t["xds_i"]] * t["xds_m"]
    XDS = xds.transpose(2, 1, 0).reshape(128, NSB * 128)
    ws = W_flat[t["ws_i"]] * t["ws_m"]  # [NSB, 128k, 128m]
    WS = ws.transpose(1, 0, 2).reshape(128, NSB * 128)
    parts = []
    for (j0, j1) in LGROUPS:
        parts.append(XDL[:, j0 * 256 : j1 * 256])
        parts.append(WL[:, int(lwoff[j0]) : int(lwoff[j1])])
    parts.append(XDS)
    parts.append(WS)
    bin_arr = np.concatenate(parts, axis=1).astype(np_dt)
    return {"bin": np.ascontiguousarray(bin_arr)}


def kernel(x, W, b):
    import ml_dtypes
    from concourse.bass_utils import run_bass_kernel_spmd

    x = np.asarray(x, np.float32)
    W = np.asarray(W, np.float32)
    b = np.asarray(b, np.float32)
    cores, bidx, slot_n, lwoff, yoff = _tables()
    np_dt = ml_dtypes.bfloat16
    x_flat = x.reshape(B, S * S)
    W_flat = W.reshape(-1)
    in_maps = [
        _pack_core(t, x_flat, W_flat, slot_n, lwoff, np_dt) for t in cores
    ]
    nc = _get_program()
    res = run_bass_kernel_spmd(
        nc, in_maps, core_ids=list(range(NCORES)), trace=TRACE
    )
    global last_results
    last_results = res
    LYC = int(yoff[-1])
    out_flat = np.zeros((B, S * S), np.float32)
    for c, t in enumerate(cores):
        yv = np.asarray(res.results[c]["yout"]).astype(np.float32)
        yv = yv.reshape(B, -1)
        fl = t["tgt_l"]
        vl = fl >= 0
        out_flat[:, fl[vl]] = yv[:, :LYC][:, vl]
        fs = t["tgt_s"].reshape(-1)
        vs = fs >= 0
        out_flat[:, fs[vs]] = yv[:, LYC:][:, vs]
    out_flat += b.reshape(-1)[bidx][None, :]
    return out_flat.reshape(B, S, S)


# revision 5
# speedup vs baseline: 1.9748x; 1.1810x over previous
"""Trainium2 Bass kernel for nn_DiagonalTraining (anti-diagonal per-diag Linear).

out[b, r, c] = sum_{r'} W[d, r - r0(d), r' - r0(d)] * x[b, r', d - r'] + bias,
with d = r + c, over the valid range of r' for diagonal d.

Strategy: shard the 511 independent diagonals across 8 cores (expert-style),
all data in bf16 (halves DMA traffic vs f32; bf16 matmuls stream 1 col/cycle
at any N, unlike f32r which needs N>=256).
  - long diagonals (n > 128): sorted by n descending, assigned round-robin so
    slot j has ~equal n on every core (SPMD shares one program). Slot matmul
    shapes use the slot max: PSUM[128b, N_j] accumulated over K-chunks
    (128, N_j-128); W shipped as [K, N_j] exact-width columns.
  - short diagonals (n <= 128): pair-packed into bins of K=128 (block-diag W),
    one matmul [K=128] x [N=128] per bin, 17 bins/core.
Stationary operand = gathered diagonal data xd^T [K, batch=128]; moving
operand = per-diagonal weights [K, N]. PSUM out = [batch=128, N] -> bf16.
Host scatters the packed outputs back to the grid and adds bias.
"""

import sys

sys.path.insert(0, "/opt/trn_rl_repo")

import numpy as np

B, S = 128, 256
D = 2 * S - 1  # 511
NCORES = 8
NSB = 17  # short-diagonal bins per core
NLJ = 32  # long-diagonal slots per core

TRACE = False  # test.py sets True to pull exec_time_ns from the NTFF profile
last_results = None


def _geom(d):
    r0 = max(0, d - S + 1)
    n = d + 1 if d < S else 2 * S - 1 - d
    return r0, n


def _slot_geom():
    """Long diags sorted by n desc, round-robin to cores; slot-wise max n."""
    longs = [(d, _geom(d)[1]) for d in range(128, 383)]
    longs.sort(key=lambda t: (-t[1], t[0]))
    per_core = [[None] * NLJ for _ in range(NCORES)]
    slot_n = [0] * NLJ
    for i, (d, n) in enumerate(longs):
        c, j = i % NCORES, i // NCORES
        per_core[c][j] = d
        slot_n[j] = max(slot_n[j], n)
    return per_core, slot_n


def _job_tables():
    """Static per-core packing tables (indices + masks + scatter targets)."""
    # ---- short bins: 129 real bins + 7 dummies = 136 = 8 * 17
    sbins = []
    for kk in range(1, 64):
        sbins.append([kk - 1, 127 - kk])
        sbins.append([511 - kk, 383 + kk])
    sbins.append([63, 447])
    sbins.append([127])
    sbins.append([383])
    sbins += [[] for _ in range(136 - len(sbins))]

    long_per_core, slot_n = _slot_geom()
    lwoff = np.concatenate([[0], np.cumsum([2 * n for n in slot_n])])
    yoff = np.concatenate([[0], np.cumsum(slot_n)])
    LWC = int(lwoff[-1])  # long W cols per core
    LYC = int(yoff[-1])  # long out cols per core

    cores = []
    for c in range(NCORES):
        my_s = sbins[c::NCORES]
        xds_i = np.zeros((NSB, 128), np.int64)
        xds_m = np.zeros((NSB, 128), np.float32)
        ws_i = np.zeros((NSB, 128, 128), np.int64)
        ws_m = np.zeros((NSB, 128, 128), np.float32)
        tgt_s = np.full((NSB, 128), -1, np.int64)
        for j, bin_ds in enumerate(my_s):
            off = 0
            for d in bin_ds:
                r0, n = _geom(d)
                i = np.arange(n)
                r = r0 + i
                col = d - r
                xds_i[j, off : off + n] = r * S + col
                xds_m[j, off : off + n] = 1.0
                # W[d, m, k] at [k, m] (k = contraction pos, m = output pos)
                ws_i[j, off : off + n, off : off + n] = (
                    d * S * S + i[None, :] * S + i[:, None]
                )
                ws_m[j, off : off + n, off : off + n] = 1.0
                tgt_s[j, off : off + n] = r * S + col
                off += n

        xdl_i = np.zeros((NLJ, 2, 128), np.int64)
        xdl_m = np.zeros((NLJ, 2, 128), np.float32)
        wl_i = np.zeros((128, LWC), np.int64)
        wl_m = np.zeros((128, LWC), np.float32)
        tgt_l = np.full(LYC, -1, np.int64)
        for j in range(NLJ):
            d = long_per_core[c][j]
            if d is None:
                continue
            r0, n = _geom(d)
            N = slot_n[j]
            m = np.arange(N)
            mv = m < n
            for ch in range(2):
                i = ch * 128 + np.arange(128)
                v = i < n
                r = r0 + np.minimum(i, n - 1)
                xdl_i[j, ch] = (r * S + (d - r)) * v
                xdl_m[j, ch] = v.astype(np.float32)
                # W chunk ch at cols [lwoff[j] + ch*N, ...): [k-part, m-col]
                wm = mv[None, :] & v[:, None]
                wi = (
                    d * S * S
                    + np.minimum(m, n - 1)[None, :] * S
                    + np.minimum(i, n - 1)[:, None]
                )
                o = int(lwoff[j]) + ch * N
                wl_i[:, o : o + N] = wi * wm
                wl_m[:, o : o + N] = wm.astype(np.float32)
            mr = r0 + m[:n]
            tgt_l[int(yoff[j]) : int(yoff[j]) + n] = mr * S + (d - mr)
        cores.append(
            dict(
                xds_i=xds_i, xds_m=xds_m, ws_i=ws_i, ws_m=ws_m, tgt_s=tgt_s,
                xdl_i=xdl_i, xdl_m=xdl_m, wl_i=wl_i, wl_m=wl_m, tgt_l=tgt_l,
            )
        )
    # bias gather: out_flat[p] += b[d, r - r0(d)] for p = r*S + c, d = r + c
    rr, cc = np.divmod(np.arange(S * S), S)
    dd = rr + cc
    r0v = np.maximum(0, dd - S + 1)
    bidx = dd * S + (rr - r0v)
    return cores, bidx, slot_n, lwoff, yoff


_TABLES = None
_PROG = None


def _tables():
    global _TABLES
    if _TABLES is None:
        _TABLES = _job_tables()
    return _TABLES


# Input-DMA groups over long slots, then one group for the short bins.
LGROUPS = [(0, 11), (11, 22), (22, 32)]
# Output-DMA groups: job-count thresholds filled in by _build_program.


def _build_program(slot_n, lwoff, yoff):
    import concourse.bass as bass
    import concourse.mybir as mybir

    f32 = mybir.dt.float32
    bf16 = mybir.dt.bfloat16
    LWC = int(lwoff[-1])
    LYC = int(yoff[-1])
    SYC = NSB * 128
    # per-group input column counts: xd cols + W cols
    gcols = []
    for (j0, j1) in LGROUPS:
        gcols.append((j1 - j0) * 256 + int(lwoff[j1] - lwoff[j0]))
    gcols.append(NSB * 128 + NSB * 128)  # shorts: xd + W
    NG = len(gcols)

    nc = bass.Bass()
    bin_t = nc.dram_tensor("bin", [128, sum(gcols)], bf16, kind="ExternalInput")
    yout = nc.dram_tensor("yout", [128, LYC + SYC], bf16, kind="ExternalOutput")

    NPS = 8  # psum banks; one pair of jobs per bank

    BT = []
    for g, gc in enumerate(gcols):
        BT.append(nc.alloc_sbuf_tensor(f"bt{g}", [128, gc], bf16).ap())
    YB = nc.alloc_sbuf_tensor("YB", [128, LYC + SYC], bf16).ap()
    PS = [
        nc.alloc_psum_tensor(f"ps{i}", [128, 512], f32).ap() for i in range(NPS)
    ]

    # unified job list: (group, kind, j, psum_col_off, yb_off, width)
    jobs = []
    for g, (j0, j1) in enumerate(LGROUPS):
        for j in range(j0, j1):
            jobs.append((g, "L", j))
    for j in range(NSB):
        jobs.append((NG - 1, "S", j))
    njobs = len(jobs)

    def job_meta(ji):
        g, kind, j = jobs[ji]
        if kind == "L":
            return int(yoff[j]), slot_n[j]
        return LYC + j * 128, 128
    # pairs: (2p, 2p+1) share PSUM bank p % 8; cast engine alternates
    npairs = (njobs + 1) // 2

    DIN = [nc.alloc_semaphore(f"din{i}") for i in range(NG)]
    P = nc.alloc_semaphore("P")  # PE job completions
    CV = nc.alloc_semaphore("CV")  # vector cast completions
    CS = nc.alloc_semaphore("CS")  # scalar cast completions
    DO = nc.alloc_semaphore("DO")  # output DMA completions (x16)

    def pair_done_counts(p_end):
        """(n_vector, n_scalar) casts among pairs [0, p_end)."""
        return (p_end + 1) // 2, p_end // 2

    # output DMA groups: (pairs_done, col_start, col_end); pair 11 starts
    # at job 22 = slot boundary of LGROUPS[1]; pair 16 at job 32 = shorts.
    og = [
        (11, 0, int(yoff[LGROUPS[1][1]])),
        (16, int(yoff[LGROUPS[1][1]]), LYC),
        (npairs, LYC, LYC + SYC),
    ]

    with nc.Block() as block:

        @block.sync
        def _(sync):
            goff = 0
            for g, gc in enumerate(gcols):
                sync.dma_start(
                    out=BT[g][:], in_=bin_t[:, goff : goff + gc]
                ).then_inc(DIN[g], 16)
                goff += gc
            sync.wait_ge(DO, 16 * len(og))

        @block.tensor
        def _(tensor):
            cur_g = -1
            for ji, (g, kind, j) in enumerate(jobs):
                if g > cur_g:
                    tensor.wait_ge(DIN[g], 16)
                    cur_g = g
                p = ji // 2
                if ji % 2 == 0 and p >= NPS:
                    # reuse bank: wait for cast of pair p - NPS
                    q = p - NPS
                    sem = CV if q % 2 == 0 else CS
                    tensor.wait_ge(sem, q // 2 + 1)
                ps = PS[p % NPS]
                po = 0 if ji % 2 == 0 else job_meta(ji - 1)[1]
                if kind == "L":
                    j0 = LGROUPS[g][0]
                    N = slot_n[j]
                    K2 = N - 128
                    xo = (j - j0) * 256
                    wo = (LGROUPS[g][1] - j0) * 256 + int(lwoff[j] - lwoff[j0])
                    bt = BT[g]
                    nc.tensor.matmul(
                        ps[:, po : po + N],
                        bt[:, xo : xo + 128],
                        bt[:, wo : wo + N],
                        start=True,
                        stop=False,
                    )
                    mm = nc.tensor.matmul(
                        ps[:, po : po + N],
                        bt[0:K2, xo + 128 : xo + 256],
                        bt[0:K2, wo + N : wo + 2 * N],
                        start=False,
                        stop=True,
                    )
                else:
                    bt = BT[NG - 1]
                    xo = j * 128
                    wo = NSB * 128 + j * 128
                    mm = nc.tensor.matmul(
                        ps[:, po : po + 128],
                        bt[:, xo : xo + 128],
                        bt[:, wo : wo + 128],
                        start=True,
                        stop=True,
                    )
                mm.then_inc(P, 1)

        @block.vector
        def _(vector):
            for p in range(0, npairs, 2):
                ja, jb = 2 * p, min(2 * p + 1, njobs - 1)
                vector.wait_ge(P, jb + 1)
                ya, na = job_meta(ja)
                width = na if jb == ja else na + job_meta(jb)[1]
                ps = PS[p % NPS]
                cp = nc.vector.tensor_copy(
                    YB[:, ya : ya + width], ps[:, 0:width]
                )
                cp.then_inc(CV, 1)

        @block.scalar
        def _(scalar):
            ogi = 0
            for p in range(1, npairs, 2):
                ja, jb = 2 * p, min(2 * p + 1, njobs - 1)
                scalar.wait_ge(P, jb + 1)
                ya, na = job_meta(ja)
                width = na if jb == ja else na + job_meta(jb)[1]
                ps = PS[p % NPS]
                cp = nc.scalar.copy(YB[:, ya : ya + width], ps[:, 0:width])
                cp.then_inc(CS, 1)
                # issue any output DMA whose pairs are all done after this one
                while ogi < len(og) and og[ogi][0] <= p + 1:
                    thr, c0, c1 = og[ogi]
                    nv, _ = pair_done_counts(thr)
                    scalar.wait_ge(CV, nv)
                    scalar.dma_start(
                        out=yout[:, c0:c1], in_=YB[:, c0:c1]
                    ).then_inc(DO, 16)
                    ogi += 1
            # trailing output groups (e.g. final pair handled by vector)
            while ogi < len(og):
                thr, c0, c1 = og[ogi]
                nv, ns = pair_done_counts(thr)
                scalar.wait_ge(CV, nv)
                scalar.wait_ge(CS, ns)
                scalar.dma_start(
                    out=yout[:, c0:c1], in_=YB[:, c0:c1]
                ).then_inc(DO, 16)
                ogi += 1

    return nc


def _get_program():
    global _PROG
    if _PROG is None:
        _, _, slot_n, lwoff, yoff = _tables()
        _PROG = _build_program(slot_n, lwoff, yoff)
    return _PROG


def _pack_core(t, x_flat, W_flat, slot_n, lwoff, np_dt):
    # long xd: [B, NLJ, 2, 128] -> [128k, NLJ*2, B] col blocks
    xdl = x_flat[:, t["xdl_i"]] * t["xdl_m"]
    XDL = xdl.transpose(3, 1, 2, 0).reshape(128, NLJ * 2 * 128)
    WL = W_flat[t["wl_i"]] * t["wl_m"]  # [128, LWC]
    # short xd: [B, NSB, 128] -> [128k, NSB, B]
    xds = x_flat[:, t["xds_i"]] * t["xds_m"]
    XDS = xds.transpose(2, 1, 0).reshape(128, NSB * 128)
    ws = W_flat[t["ws_i"]] * t["ws_m"]  # [NSB, 128k, 128m]
    WS = ws.transpose(1, 0, 2).reshape(128, NSB * 128)
    parts = []
    for (j0, j1) in LGROUPS:
        parts.append(XDL[:, j0 * 256 : j1 * 256])
        parts.append(WL[:, int(lwoff[j0]) : int(lwoff[j1])])
    parts.append(XDS)
    parts.append(WS)
    bin_arr = np.concatenate(parts, axis=1).astype(np_dt)
    return {"bin": np.ascontiguousarray(bin_arr)}


def kernel(x, W, b):
    import ml_dtypes
    from concourse.bass_utils import run_bass_kernel_spmd

    x = np.asarray(x, np.float32)
    W = np.asarray(W, np.float32)
    b = np.asarray(b, np.float32)
    cores, bidx, slot_n, lwoff, yoff = _tables()
    np_dt = ml_dtypes.bfloat16
    x_flat = x.reshape(B, S * S)
    W_flat = W.reshape(-1)
    in_maps = [
        _pack_core(t, x_flat, W_flat, slot_n, lwoff, np_dt) for t in cores
    ]
    nc = _get_program()
    res = run_bass_kernel_spmd(
        nc, in_maps, core_ids=list(range(NCORES)), trace=TRACE
    )
    global last_results
    last_results = res
    LYC = int(yoff[-1])
    out_flat = np.zeros((B, S * S), np.float32)
    for c, t in enumerate(cores):
        yv = np.asarray(res.results[c]["yout"]).astype(np.float32)
        yv = yv.reshape(B, -1)
        fl = t["tgt_l"]
        vl = fl >= 0
        out_flat[:, fl[vl]] = yv[:, :LYC][:, vl]
        fs = t["tgt_s"].reshape(-1)
        vs = fs >= 0
        out_flat[:, fs[vs]] = yv[:, LYC:][:, vs]
    out_flat += b.reshape(-1)[bidx][None, :]
    return out_flat.reshape(B, S, S)


# revision 7
# speedup vs baseline: 2.0336x; 1.0298x over previous
"""Trainium2 Bass kernel for nn_DiagonalTraining (anti-diagonal per-diag Linear).

out[b, r, c] = sum_{r'} W[d, r - r0(d), r' - r0(d)] * x[b, r', d - r'] + bias,
with d = r + c, over the valid range of r' for diagonal d.

Strategy: shard the 511 independent diagonals across 8 cores (expert-style),
all data in bf16.
  - long diagonals (n > 128): sorted by n descending, assigned round-robin so
    slot j has ~equal n on every core (SPMD shares one program). Slot matmul
    shapes use the slot max N_j: PSUM[128b, N_j] accumulated over K-chunks
    (128, K2_j = N_j - 128); W shipped as [K, N_j] exact-width columns.
    Chunk2 K-partitions are vertically packed: slots with small K2 "ride" in
    the dead partition rows of a carrier slot's chunk2 block (rider matmuls
    use tile_position=(64|96, 0)), eliminating most zero-padding traffic.
  - short diagonals (n <= 128): pair-packed into bins of K=128 (block-diag
    W), one matmul [K=128] x [N=128] per bin, 17 bins/core.
Stationary operand = gathered diagonal data xd^T [K, batch=128]; moving
operand = per-diagonal weights [K, N]. Jobs are paired two-per-PSUM-bank;
PSUM->SBUF bf16 casts alternate between Vector and Scalar engines; input
DMAs ride the sync-engine HWDGE ring while output DMAs use the scalar ring.
Host scatters the packed outputs back to the grid and adds bias.
"""

import sys

sys.path.insert(0, "/opt/trn_rl_repo")

import numpy as np

B, S = 128, 256
D = 2 * S - 1  # 511
NCORES = 8
NSB = 17  # short-diagonal bins per core
NLJ = 32  # long-diagonal slots per core

TRACE = False  # test.py sets True to pull exec_time_ns from the NTFF profile
last_results = None

# chunk2 vertical packing: rider slot -> (carrier slot, partition base)
CARRIER_OF = {}
for _i in range(8):
    CARRIER_OF[24 + _i] = (8 + _i, 96)
for _i in range(4):
    CARRIER_OF[17 + 2 * _i] = (16 + 2 * _i, 64)

# input-DMA groups over long slots, then one group for the short bins
LGROUPS = [(0, 11), (11, 22), (22, 32)]


def _geom(d):
    r0 = max(0, d - S + 1)
    n = d + 1 if d < S else 2 * S - 1 - d
    return r0, n


def _slot_geom():
    """Long diags sorted by n desc, round-robin to cores; slot-wise max n."""
    longs = [(d, _geom(d)[1]) for d in range(128, 383)]
    longs.sort(key=lambda t: (-t[1], t[0]))
    per_core = [[None] * NLJ for _ in range(NCORES)]
    slot_n = [0] * NLJ
    for i, (d, n) in enumerate(longs):
        c, j = i % NCORES, i // NCORES
        per_core[c][j] = d
        slot_n[j] = max(slot_n[j], n)
    return per_core, slot_n


def _layout(slot_n):
    """Shared column layout: xd blocks, W col offsets, group extents."""
    xd_blocks = []  # (slot, chunk) in slot order
    for j in range(NLJ):
        xd_blocks.append((j, 0))
        if j not in CARRIER_OF:
            xd_blocks.append((j, 1))
    xd_pos = {bc: i for i, bc in enumerate(xd_blocks)}
    lw = [slot_n[j] if j in CARRIER_OF else 2 * slot_n[j] for j in range(NLJ)]
    lwoff = np.concatenate([[0], np.cumsum(lw)]).astype(np.int64)
    yoff = np.concatenate([[0], np.cumsum(slot_n)]).astype(np.int64)
    # per group: (slot range, xd block index range, group col count)
    groups = []
    for (j0, j1) in LGROUPS:
        b0 = xd_pos[(j0, 0)]
        b1 = xd_pos[(j1, 0)] if j1 < NLJ else len(xd_blocks)
        nx = (b1 - b0) * 128
        gc = nx + int(lwoff[j1] - lwoff[j0])
        groups.append((j0, j1, b0, b1, nx, gc))
    return xd_blocks, xd_pos, lwoff, yoff, groups


def _job_tables():
    """Static per-core packing tables (indices + masks + scatter targets)."""
    # ---- short bins: 129 real bins + 7 dummies = 136 = 8 * 17
    sbins = []
    for kk in range(1, 64):
        sbins.append([kk - 1, 127 - kk])
        sbins.append([511 - kk, 383 + kk])
    sbins.append([63, 447])
    sbins.append([127])
    sbins.append([383])
    sbins += [[] for _ in range(136 - len(sbins))]

    long_per_core, slot_n = _slot_geom()
    xd_blocks, xd_pos, lwoff, yoff, groups = _layout(slot_n)
    NXB = len(xd_blocks)
    LWC = int(lwoff[-1])
    LYC = int(yoff[-1])

    cores = []
    for c in range(NCORES):
        my_s = sbins[c::NCORES]
        xds_i = np.zeros((NSB, 128), np.int64)
        xds_m = np.zeros((NSB, 128), np.float32)
        ws_i = np.zeros((NSB, 128, 128), np.int64)
        ws_m = np.zeros((NSB, 128, 128), np.float32)
        tgt_s = np.full((NSB, 128), -1, np.int64)
        for j, bin_ds in enumerate(my_s):
            off = 0
            for d in bin_ds:
                r0, n = _geom(d)
                i = np.arange(n)
                r = r0 + i
                col = d - r
                xds_i[j, off : off + n] = r * S + col
                xds_m[j, off : off + n] = 1.0
                # W[d, m, k] at [k, m] (k = contraction pos, m = output pos)
                ws_i[j, off : off + n, off : off + n] = (
                    d * S * S + i[None, :] * S + i[:, None]
                )
                ws_m[j, off : off + n, off : off + n] = 1.0
                tgt_s[j, off : off + n] = r * S + col
                off += n

        xdl_i = np.zeros((NXB, 128), np.int64)
        xdl_m = np.zeros((NXB, 128), np.float32)
        wl_i = np.zeros((128, LWC), np.int64)
        wl_m = np.zeros((128, LWC), np.float32)
        tgt_l = np.full(LYC, -1, np.int64)

        for j in range(NLJ):
            d = long_per_core[c][j]
            if d is None:
                continue
            r0, n = _geom(d)
            N = slot_n[j]
            m = np.arange(N)
            mv = m < n
            # chunk1: xd block (j, 0), W cols [lwoff[j], +N)
            blk = xd_pos[(j, 0)]
            k = np.arange(128)
            v = k < n
            r = r0 + np.minimum(k, n - 1)
            xdl_i[blk] = (r * S + (d - r)) * v
            xdl_m[blk] = v.astype(np.float32)
            o = int(lwoff[j])
            wm = mv[None, :] & v[:, None]
            wl_i[:, o : o + N] = (
                d * S * S
                + np.minimum(m, n - 1)[None, :] * S
                + np.minimum(k, n - 1)[:, None]
            ) * wm
            wl_m[:, o : o + N] = wm.astype(np.float32)
            # chunk2: either own block or rider rows in carrier's block
            if j in CARRIER_OF:
                cj, base = CARRIER_OF[j]
                blk2 = xd_pos[(cj, 1)]
                rows = np.arange(base, 128)
                k2 = 128 + (rows - base)
                o2 = int(lwoff[cj]) + slot_n[cj]
                N2 = N  # rider W cols: first N of carrier chunk2 block
            else:
                blk2 = xd_pos[(j, 1)]
                rows = np.arange(0, 128)
                k2 = 128 + rows
                o2 = o + N
                N2 = N
            v2 = k2 < n
            r2 = r0 + np.minimum(k2, n - 1)
            xdl_i[blk2, rows] = (r2 * S + (d - r2)) * v2
            xdl_m[blk2, rows] = v2.astype(np.float32)
            wm2 = mv[None, :N2] & v2[:, None]
            wl_i[np.ix_(rows, np.arange(o2, o2 + N2))] = (
                d * S * S
                + np.minimum(m[:N2], n - 1)[None, :] * S
                + np.minimum(k2, n - 1)[:, None]
            ) * wm2
            wl_m[np.ix_(rows, np.arange(o2, o2 + N2))] = wm2.astype(np.float32)
            mr = r0 + m[:n]
            tgt_l[int(yoff[j]) : int(yoff[j]) + n] = mr * S + (d - mr)
        cores.append(
            dict(
                xds_i=xds_i, xds_m=xds_m, ws_i=ws_i, ws_m=ws_m, tgt_s=tgt_s,
                xdl_i=xdl_i, xdl_m=xdl_m, wl_i=wl_i, wl_m=wl_m, tgt_l=tgt_l,
            )
        )
    # bias gather: out_flat[p] += b[d, r - r0(d)] for p = r*S + c, d = r + c
    rr, cc = np.divmod(np.arange(S * S), S)
    dd = rr + cc
    r0v = np.maximum(0, dd - S + 1)
    bidx = dd * S + (rr - r0v)
    return cores, bidx, slot_n, lwoff, yoff, groups, xd_pos


_TABLES = None
_PROG = None


def _tables():
    global _TABLES
    if _TABLES is None:
        _TABLES = _job_tables()
    return _TABLES


def _build_program(slot_n, lwoff, yoff, groups, xd_pos):
    import concourse.bass as bass
    import concourse.mybir as mybir

    f32 = mybir.dt.float32
    bf16 = mybir.dt.bfloat16
    LYC = int(yoff[-1])
    SYC = NSB * 128
    gcols = [g[5] for g in groups] + [NSB * 256]  # shorts: xd + W
    NG = len(gcols)
    group_of_slot = {}
    for g, (j0, j1, b0, b1, nx, gc) in enumerate(groups):
        for j in range(j0, j1):
            group_of_slot[j] = g

    nc = bass.Bass(enable_partition_id=False)
    bin_t = nc.dram_tensor("bin", [128, sum(gcols)], bf16, kind="ExternalInput")
    yout = nc.dram_tensor("yout", [128, LYC + SYC], bf16, kind="ExternalOutput")

    NPS = 8  # psum banks; one pair of jobs per bank

    BT = [
        nc.alloc_sbuf_tensor(f"bt{g}", [128, gc], bf16).ap()
        for g, gc in enumerate(gcols)
    ]
    YB = nc.alloc_sbuf_tensor("YB", [128, LYC + SYC], bf16).ap()
    PS = [
        nc.alloc_psum_tensor(f"ps{i}", [128, 512], f32).ap() for i in range(NPS)
    ]

    def slot_aps(j):
        """(bt, xd1_off, w1_off, bt2, rows_base, xd2_off, w2_off) for slot j."""
        g = group_of_slot[j]
        j0, j1, b0, b1, nx, gc = groups[g]
        xo1 = (xd_pos[(j, 0)] - b0) * 128
        wo1 = nx + int(lwoff[j] - lwoff[j0])
        if j in CARRIER_OF:
            cj, base = CARRIER_OF[j]
            g2 = group_of_slot[cj]
            c0, c1, cb0, cb1, cnx, cgc = groups[g2]
            xo2 = (xd_pos[(cj, 1)] - cb0) * 128
            wo2 = cnx + int(lwoff[cj] - lwoff[c0]) + slot_n[cj]
            return BT[g], xo1, wo1, BT[g2], base, xo2, wo2
        xo2 = (xd_pos[(j, 1)] - b0) * 128
        wo2 = wo1 + slot_n[j]
        return BT[g], xo1, wo1, BT[g], 0, xo2, wo2

    # unified job list: (group, kind, j)
    jobs = []
    for g, (j0, j1, *_rest) in enumerate(groups):
        for j in range(j0, j1):
            jobs.append((g, "L", j))
    for j in range(NSB):
        jobs.append((NG - 1, "S", j))
    njobs = len(jobs)
    npairs = (njobs + 1) // 2

    def job_meta(ji):
        g, kind, j = jobs[ji]
        if kind == "L":
            return int(yoff[j]), slot_n[j]
        return LYC + j * 128, 128

    DIN = [nc.alloc_semaphore(f"din{i}") for i in range(NG)]
    P = nc.alloc_semaphore("P")  # PE job completions
    CV = nc.alloc_semaphore("CV")  # vector cast completions
    CS = nc.alloc_semaphore("CS")  # scalar cast completions
    DO = nc.alloc_semaphore("DO")  # output DMA completions (x16)

    def pair_done_counts(p_end):
        """(n_vector, n_scalar) casts among pairs [0, p_end)."""
        return (p_end + 1) // 2, p_end // 2

    # output DMA groups: (pairs_done, col_start, col_end); boundaries on
    # even job indices. Last group kept small to shrink the exit tail.
    def job_col(ji):
        return job_meta(ji)[0] if ji < njobs else LYC + SYC

    og = []
    for p0, p1 in [(0, 6), (6, 11), (11, 16), (16, 21), (21, npairs)]:
        og.append((p1, job_col(2 * p0), job_col(2 * p1)))

    with nc.Block(no_gpsimd_drain=True) as block:

        @block.sync
        def _(sync):
            goff = 0
            for g, gc in enumerate(gcols):
                sync.dma_start(
                    out=BT[g][:], in_=bin_t[:, goff : goff + gc]
                ).then_inc(DIN[g], 16)
                goff += gc
            sync.wait_ge(DO, 16 * len(og))

        @block.tensor
        def _(tensor):
            cur_g = -1
            for ji, (g, kind, j) in enumerate(jobs):
                if g > cur_g:
                    tensor.wait_ge(DIN[g], 16)
                    cur_g = g
                p = ji // 2
                if ji % 2 == 0 and p >= NPS:
                    q = p - NPS
                    sem = CV if q % 2 == 0 else CS
                    tensor.wait_ge(sem, q // 2 + 1)
                ps = PS[p % NPS]
                po = 0 if ji % 2 == 0 else job_meta(ji - 1)[1]
                if kind == "L":
                    N = slot_n[j]
                    K2 = N - 128
                    bt, xo1, wo1, bt2, base, xo2, wo2 = slot_aps(j)
                    nc.tensor.matmul(
                        ps[:, po : po + N],
                        bt[:, xo1 : xo1 + 128],
                        bt[:, wo1 : wo1 + N],
                        start=True,
                        stop=False,
                    )
                    mm = nc.tensor.matmul(
                        ps[:, po : po + N],
                        bt2[base : base + K2, xo2 : xo2 + 128],
                        bt2[base : base + K2, wo2 : wo2 + N],
                        start=False,
                        stop=True,
                        tile_position=(base, 0),
                    )
                else:
                    bt = BT[NG - 1]
                    xo = j * 128
                    wo = NSB * 128 + j * 128
                    mm = nc.tensor.matmul(
                        ps[:, po : po + 128],
                        bt[:, xo : xo + 128],
                        bt[:, wo : wo + 128],
                        start=True,
                        stop=True,
                    )
                mm.then_inc(P, 1)

        @block.vector
        def _(vector):
            for p in range(0, npairs, 2):
                ja, jb = 2 * p, min(2 * p + 1, njobs - 1)
                vector.wait_ge(P, jb + 1)
                ya, na = job_meta(ja)
                width = na if jb == ja else na + job_meta(jb)[1]
                ps = PS[p % NPS]
                cp = nc.vector.tensor_copy(
                    YB[:, ya : ya + width], ps[:, 0:width]
                )
                cp.then_inc(CV, 1)

        @block.scalar
        def _(scalar):
            ogi = 0
            for p in range(1, npairs, 2):
                ja, jb = 2 * p, min(2 * p + 1, njobs - 1)
                scalar.wait_ge(P, jb + 1)
                ya, na = job_meta(ja)
                width = na if jb == ja else na + job_meta(jb)[1]
                ps = PS[p % NPS]
                cp = nc.scalar.copy(YB[:, ya : ya + width], ps[:, 0:width])
                cp.then_inc(CS, 1)
                while ogi < len(og) and og[ogi][0] <= p + 1:
                    thr, c0, c1 = og[ogi]
                    nv, _ns = pair_done_counts(thr)
                    scalar.wait_ge(CV, nv)
                    scalar.dma_start(
                        out=yout[:, c0:c1], in_=YB[:, c0:c1]
                    ).then_inc(DO, 16)
                    ogi += 1
            while ogi < len(og):
                thr, c0, c1 = og[ogi]
                nv, ns = pair_done_counts(thr)
                scalar.wait_ge(CV, nv)
                scalar.wait_ge(CS, ns)
                scalar.dma_start(
                    out=yout[:, c0:c1], in_=YB[:, c0:c1]
                ).then_inc(DO, 16)
                ogi += 1

    return nc


def _get_program():
    global _PROG
    if _PROG is None:
        _, _, slot_n, lwoff, yoff, groups, xd_pos = _tables()
        _PROG = _build_program(slot_n, lwoff, yoff, groups, xd_pos)
    return _PROG


def _pack_core(t, x_flat, W_flat, lwoff, groups, np_dt):
    # long xd: [B, NXB, 128] -> [128k, NXB, B] col blocks
    xdl = x_flat[:, t["xdl_i"]] * t["xdl_m"]
    NXB = t["xdl_i"].shape[0]
    XDL = xdl.transpose(2, 1, 0).reshape(128, NXB * 128)
    WL = W_flat[t["wl_i"]] * t["wl_m"]  # [128, LWC]
    # short xd: [B, NSB, 128] -> [128k, NSB, B]
    xds = x_flat[:, t["xds_i"]] * t["xds_m"]
    XDS = xds.transpose(2, 1, 0).reshape(128, NSB * 128)
    ws = W_flat[t["ws_i"]] * t["ws_m"]  # [NSB, 128k, 128m]
    WS = ws.transpose(1, 0, 2).reshape(128, NSB * 128)
    parts = []
    for (j0, j1, b0, b1, nx, gc) in groups:
        parts.append(XDL[:, b0 * 128 : b1 * 128])
        parts.append(WL[:, int(lwoff[j0]) : int(lwoff[j1])])
    parts.append(XDS)
    parts.append(WS)
    bin_arr = np.concatenate(parts, axis=1).astype(np_dt)
    return {"bin": np.ascontiguousarray(bin_arr)}


def kernel(x, W, b):
    import ml_dtypes
    from concourse.bass_utils import run_bass_kernel_spmd

    x = np.asarray(x, np.float32)
    W = np.asarray(W, np.float32)
    b = np.asarray(b, np.float32)
    cores, bidx, slot_n, lwoff, yoff, groups, xd_pos = _tables()
    np_dt = ml_dtypes.bfloat16
    x_flat = x.reshape(B, S * S)
    W_flat = W.reshape(-1)
    in_maps = [
        _pack_core(t, x_flat, W_flat, lwoff, groups, np_dt) for t in cores
    ]
    nc = _get_program()
    res = run_bass_kernel_spmd(
        nc, in_maps, core_ids=list(range(NCORES)), trace=TRACE
    )
    global last_results
    last_results = res
    LYC = int(yoff[-1])
    out_flat = np.zeros((B, S * S), np.float32)
    for c, t in enumerate(cores):
        yv = np.asarray(res.results[c]["yout"]).astype(np.float32)
        yv = yv.reshape(B, -1)
        fl = t["tgt_l"]
        vl = fl >= 0
        out_flat[:, fl[vl]] = yv[:, :LYC][:, vl]
        fs = t["tgt_s"].reshape(-1)
        vs = fs >= 0
        out_flat[:, fs[vs]] = yv[:, LYC:][:, vs]
    out_flat += b.reshape(-1)[bidx][None, :]
    return out_flat.reshape(B, S, S)
